# revision 15
# baseline (speedup 1.0000x reference)
"""Trainium2 Bass kernel for nn_C2M_24378234372461.

Data-parallel over batch (8 samples on 8 NeuronCores). BatchNorm batch
statistics are exchanged with two small collectives whose setup cost is
absorbed by a front-loaded dummy collective. Matmuls run in bf16.
Self-contained: builds + compiles the Bass program on first call.
"""
import sys

for _p in ("/opt/trn_rl_repo",):
    if _p not in sys.path:
        sys.path.append(_p)

import numpy as np
import concourse.bacc as bacc
import concourse.bass as bass
import concourse.mybir as mybir
import concourse.tile as tile
import concourse.masks as masks
from concourse.bass_utils import run_bass_kernel_spmd

f32 = mybir.dt.float32
f32r = mybir.dt.float32r
bf16 = mybir.dt.bfloat16
i32 = mybir.dt.int32
AF = mybir.ActivationFunctionType
AX = mybir.AxisListType
ALU = mybir.AluOpType

N_CORES = 8
B = 8
C2, H2, W2 = 128, 44, 44
C3, H3, W3 = 256, 22, 22
C4, H4, W4 = 512, 11, 11
HW2 = H2 * W2            # 1936
HW3 = H3 * W3            # 484
HW4 = H4 * W4            # 121
PG2 = 46 * 46            # 2116 padded grid scale-2
PG3 = 24 * 24            # 576  padded grid scale-3
XP2_W = PG2 + 96         # flat buffer + tail for overcompute reads (2212)
XP3_W = PG3 + 50         # 626 per cin tile
EPS = 1e-5


def _view2d(ap, width):
    """[p, (rows width)] view of a flat AP."""
    return ap.rearrange("p (r w) -> p r w", w=width)


def build(shared, DEBUG=False):
    nc = bacc.Bacc("TRN2", target_bir_lowering=False, debug=False,
                   num_devices=N_CORES)

    # ---------------- DRAM I/O ----------------
    d_xp4s = nc.dram_tensor("xp4s", [4, 128, 9 * HW4], bf16, kind="ExternalInput")
    d_xp3 = nc.dram_tensor("xp3", [128, 2 * XP3_W], bf16, kind="ExternalInput")
    d_xp2 = nc.dram_tensor("xp2", [128, XP2_W], f32r, kind="ExternalInput")
    d_xp2b = nc.dram_tensor("xp2b", [128, XP2_W], bf16, kind="ExternalInput")
    d_w4qk = nc.inline_tensor(shared["w4qk"], "w4qk")
    d_wn3 = nc.inline_tensor(shared["wn3"], "wn3")
    d_wn2 = nc.inline_tensor(shared["wn2"], "wn2")
    d_w2r = nc.inline_tensor(shared["w2r"], "w2r")
    d_w13 = nc.inline_tensor(shared["w13"], "w13")
    d_w12 = nc.inline_tensor(shared["w12"], "w12")
    d_bn4 = nc.inline_tensor(shared["bn4"], "bn4")
    d_vecs = nc.inline_tensor(shared["vecs"], "vecs")
    d_ones = nc.inline_tensor(shared["onesd"], "onesd")
    d_out = nc.dram_tensor("out", [128, HW2], f32, kind="ExternalOutput")
    if DEBUG:
        dbg_zqkT = nc.dram_tensor("dbg_zqkT", [121, 512], f32, kind="ExternalOutput")
        dbg_gath1 = nc.dram_tensor("dbg_gath1", [8, 1024], f32, kind="ExternalOutput")
        dbg_var = nc.dram_tensor("dbg_var", [1, 512], f32, kind="ExternalOutput")
        dbg_r4qkT = nc.dram_tensor("dbg_r4qkT", [128, 512], f32, kind="ExternalOutput")
        dbg_r3 = nc.dram_tensor("dbg_r3", [128, 968], f32, kind="ExternalOutput")
        dbg_s43 = nc.dram_tensor("dbg_s43", [121, 1], f32, kind="ExternalOutput")
        dbg_q3 = nc.dram_tensor("dbg_q3", [128, 484], f32, kind="ExternalOutput")
        dbg_r2 = nc.dram_tensor("dbg_r2", [128, 1936], f32, kind="ExternalOutput")
        dbg_s2 = nc.dram_tensor("dbg_s2", [128, 1938], f32, kind="ExternalOutput")
        dbg_q2 = nc.dram_tensor("dbg_q2", [128, 1934], f32, kind="ExternalOutput")
        dbg_k2T = nc.dram_tensor("dbg_k2T", [128, 2048], f32, kind="ExternalOutput")
        dbg_y = nc.dram_tensor("dbg_y", [128, 1936], f32, kind="ExternalOutput")
        dbg_gs2 = nc.dram_tensor("dbg_gs2", [128, 2], f32, kind="ExternalOutput")
        dbg_AB = nc.dram_tensor("dbg_AB", [128, 2], f32, kind="ExternalOutput")

    # collective bounce buffers
    cc0_in = nc.dram_tensor("cc0_in", [1, 8], f32r)
    cc0_out = nc.dram_tensor("cc0_out", [1, 8], f32r, addr_space="Shared")
    cc0b_out = nc.dram_tensor("cc0b_out", [1, 8], f32r, addr_space="Shared")
    cc0c_out = nc.dram_tensor("cc0c_out", [1, 8], f32r, addr_space="Shared")
    cc1_in = nc.dram_tensor("cc1_in", [1, 1024], f32r)
    cc1_out = nc.dram_tensor("cc1_out", [1, 1024], f32r, addr_space="Shared")
    cc2_in = nc.dram_tensor("cc2_in", [128, 2], f32r)
    cc2_out = nc.dram_tensor("cc2_out", [128, 2], f32r, addr_space="Shared")
    RG = [list(range(N_CORES))]

    with tile.TileContext(nc) as tc:
        _build_body(nc, tc, locals())
    nc.compile()
    return nc


def _build_body(nc, tc, d):
    from contextlib import ExitStack

    ctx = ExitStack()
    with ctx:
        # Dummy collectives: first ops on the CC queue. The first absorbs
        # the one-time collective setup / rendezvous cost in the background
        # while the tensor engine computes; the second confirms the warm
        # path so the real stats collective behind them is fast.
        nc.gpsimd.collective_compute(
            "AllReduce", ALU.add, replica_groups=d["RG"],
            ins=[d["cc0_in"][:].opt()], outs=[d["cc0_out"][:].opt()])
        const = ctx.enter_context(tc.tile_pool(name="const", bufs=1))
        acts = ctx.enter_context(tc.tile_pool(name="acts", bufs=1))
        scr = ctx.enter_context(tc.tile_pool(name="scr", bufs=3))
        attp = ctx.enter_context(tc.tile_pool(name="attp", bufs=3))
        ps_tmp = ctx.enter_context(tc.tile_pool(name="ps_tmp", bufs=2, space="PSUM"))
        ps_big = ctx.enter_context(tc.tile_pool(name="ps_big", bufs=1, space="PSUM"))

        _tmp_i = [0]
        CH2 = [(0, 506), (506, 506), (1012, 506), (1518, 506)]

        def tmp_ps(p, n):
            _tmp_i[0] += 1
            return ps_tmp.tile([p, n], f32, tag="tmp", name=f"tps{_tmp_i[0]}")

        # ------------- constants / weights (persistent) -------------
        ident = const.tile([128, 128], f32)
        masks.make_identity(nc, ident[:])
        ones_sb = const.tile([128, 256], f32r)
        nc.sync.dma_start(ones_sb[:], d["d_ones"][:].bitcast(f32r))
        vecs = const.tile([128, 10], f32)
        nc.sync.dma_start(vecs[:], d["d_vecs"][:])
        bn4gb = const.tile([1, 1024], f32)
        wn3 = const.tile([128, 4608], bf16)
        wn2 = const.tile([128, 1152], bf16)
        w2r = const.tile([128, 1152], bf16)
        w13 = const.tile([128, 1536], bf16)
        w12 = const.tile([128, 1280], bf16)

        # preload the sqrt activation table during startup so BN1's Sqrt
        # causes no table swap on the critical path
        eps1 = const.tile([1, 1], f32)
        nc.vector.memset(eps1[:], EPS)
        sqd = const.tile([1, 1], f32)
        nc.scalar.activation(sqd[:], eps1[:], AF.Sqrt)

        # ------------- persistent activations -------------
        xp2 = acts.tile([128, XP2_W], f32r)
        xp2b = acts.tile([128, XP2_W], bf16)
        r2 = acts.tile([128, HW2], bf16)
        q3 = acts.tile([128, HW3], bf16)
        k3T = acts.tile([128, 512], bf16)
        s2pad = acts.tile([128, HW2 + 2], bf16)
        q2 = acts.tile([128, 1934], bf16)
        k2 = acts.tile([128, 1934], f32)
        k2T = acts.tile([128, 2048], bf16)
        r2fpad = acts.tile([128, XP2_W], bf16)
        y_sb = acts.tile([128, HW2], f32)

        # zero the padding borders (replaces the zeros DMA)
        nc.vector.memset(s2pad[:, 0:1], 0.0)
        nc.vector.memset(s2pad[:, HW2 + 1: HW2 + 2], 0.0)
        g2 = _view2d(r2fpad[:, :PG2], 46)
        nc.vector.memset(g2[:, 0:1, :], 0.0)
        nc.vector.memset(g2[:, 45:46, :], 0.0)
        nc.vector.memset(g2[:, 1:45, 0:1], 0.0)
        nc.vector.memset(g2[:, 1:45, 45:46], 0.0)
        nc.vector.memset(r2fpad[:, PG2:], 0.0)

        # ============ PHASE A: scale-4 + BN1 + att43 + s3 + q3/k3 ============
        with tc.tile_pool(name="s1", bufs=1) as s1p, \
             tc.tile_pool(name="s1s", bufs=1) as s1s:
            xp3 = s1p.tile([128, 2 * XP3_W], bf16)

            # z^T = conv(x4) for q|k stacked: [121, 512]
            zT = tmp_ps(121, 512)
            with tc.tile_pool(name="s1w", bufs=3) as s1w, \
                 tc.tile_pool(name="s1x", bufs=1) as s1x:
                for t in range(4):
                    xc = s1x.tile([128, 9 * HW4], bf16, tag="x4c",
                                  name=f"x4c{t}")
                    nc.sync.dma_start(xc[:, :545], d["d_xp4s"][t][:, :545])
                    nc.sync.dma_start(xc[:, 545:], d["d_xp4s"][t][:, 545:])
                    for tp2 in range(3):
                        wc = s1w.tile([128, 1536], bf16, tag="w4c",
                                      name=f"w4c{t}_{tp2}")
                        for wsp in range(3):
                            nc.sync.dma_start(
                                wc[:, wsp * 512:(wsp + 1) * 512],
                                d["d_w4qk"][t][:, tp2 * 1536 + wsp * 512:
                                               tp2 * 1536 + (wsp + 1) * 512])
                        for tj in range(3):
                            tap = tp2 * 3 + tj
                            nc.tensor.matmul(
                                zT[:],
                                xc[:, tap * HW4:(tap + 1) * HW4],
                                wc[:, tj * 512:(tj + 1) * 512],
                                start=(t == 0 and tap == 0),
                                stop=(t == 3 and tap == 8))
            # stats -> collective, triggered as early as possible
            zqkT = s1p.tile([121, 512], f32r)
            nc.vector.tensor_copy(zqkT[:], zT[:])
            if d.get("DEBUG"):
                nc.sync.dma_start(d["dbg_zqkT"][:], zqkT[:].bitcast(f32))
            zsq = s1p.tile([121, 512], f32r, tag="zt1", name="zsq")
            nc.vector.tensor_mul(zsq[:], zqkT[:], zqkT[:])
            stats_ps = tmp_ps(1, 1024)
            nc.tensor.matmul(stats_ps[:, :512], ones_sb[:121, :1], zqkT[:],
                             start=True, stop=True)
            nc.tensor.matmul(stats_ps[:, 512:], ones_sb[:121, :1], zsq[:],
                             start=True, stop=True)
            stats1 = s1p.tile([1, 1024], f32r)
            nc.vector.tensor_copy(stats1[:], stats_ps[:])
            nc.sync.dma_start(d["cc1_in"][:], stats1[:])
            nc.gpsimd.collective_compute(
                "AllReduce", ALU.add, replica_groups=d["RG"],
                ins=[d["cc1_in"][:].opt()], outs=[d["cc1_out"][:].opt()])
            gath1 = s1p.tile([1, 1024], f32r)
            nc.sync.dma_start(gath1[:], d["cc1_out"][:])
            # PE warmup burst (HAM unthrottle) bridges startup DMA waits
            for wi in range(8):
                wp = tmp_ps(128, 256)
                nc.tensor.matmul(wp[:], ones_sb[:, :128], ones_sb[:],
                                 start=True, stop=True)
            # deferred input DMAs (after the stage-1 critical path)
            nc.sync.dma_start(xp3[:, :XP3_W], d["d_xp3"][:, :XP3_W])
            nc.sync.dma_start(xp3[:, XP3_W:], d["d_xp3"][:, XP3_W:])
            for wsp in range(4):
                nc.sync.dma_start(wn3[:, wsp * 1152:(wsp + 1) * 1152],
                                  d["d_wn3"][:, wsp * 1152:(wsp + 1) * 1152])
            nc.sync.dma_start(xp2b[:, :1106], d["d_xp2b"][:, :1106])
            nc.sync.dma_start(xp2b[:, 1106:], d["d_xp2b"][:, 1106:])
            nc.sync.dma_start(wn2[:], d["d_wn2"][:])
            nc.sync.dma_start(bn4gb[:, :512], d["d_bn4"][0:1, :])
            nc.sync.dma_start(bn4gb[:, 512:], d["d_bn4"][1:2, :])
            nc.sync.dma_start(w13[:], d["d_w13"][:])
            nc.sync.dma_start(w12[:], d["d_w12"][:])
            nc.sync.dma_start(w2r[:], d["d_w2r"][:])
            nc.sync.dma_start(xp2[:, :1106], d["d_xp2"][:, :1106])
            nc.sync.dma_start(xp2[:, 1106:], d["d_xp2"][:, 1106:])

            # r3 = conv_n3(x3) + bias  [2ct][128, 484] — independent of the
            # collective; fills its latency window on the tensor queue
            r3 = s1p.tile([128, 2 * HW3], bf16)
            for ct in range(2):
                for ch in range(2):          # padded-grid chunks of 288 (12 rows)
                    pc = tmp_ps(128, 288)
                    for kt in range(2):
                        for tap in range(9):
                            dy, dx = tap // 3, tap % 3
                            off = kt * XP3_W + ch * 288 + dy * 24 + dx
                            nc.tensor.matmul(
                                pc[:], wn3[:, (kt * 18 + ct * 9 + tap) * 128:
                                           (kt * 18 + ct * 9 + tap + 1) * 128],
                                xp3[:, off: off + 288],
                                start=(kt == 0 and tap == 0),
                                stop=(kt == 1 and tap == 8))
                    r0 = ch * 12
                    nr = min(12, 22 - r0)
                    src = _view2d(pc[:, :nr * 24], 24)[:, :, :22]
                    nc.scalar.activation(
                        r3[:, ct * HW3 + r0 * 22:
                           ct * HW3 + r0 * 22 + nr * 22],
                        src, AF.Identity, bias=vecs[:, ct:ct + 1])
            if d.get("DEBUG"):
                nc.sync.dma_start(d["dbg_r3"][:], r3[:].bitcast(f32))

            # r2 conv (also collective-independent)
            for ci, (st, sz) in enumerate(CH2):
                pc = tmp_ps(128, sz)
                for tap in range(9):
                    dy, dx = tap // 3, tap % 3
                    nc.tensor.matmul(
                        pc[:], wn2[:, tap * 128:(tap + 1) * 128],
                        xp2b[:, st + dy * 46 + dx: st + dy * 46 + dx + sz],
                        start=(tap == 0), stop=(tap == 8))
                r0 = st // 46
                src = _view2d(pc[:, :11 * 46], 46)[:, :, :44]
                nc.scalar.activation(
                    r2[:, r0 * 44: r0 * 44 + 11 * 44], src,
                    AF.Identity, bias=vecs[:, 2:3])
            if d.get("DEBUG"):
                nc.sync.dma_start(d["dbg_r2"][:], r2[:].bitcast(f32))

            # BN affine: A = g * rsqrt(var+eps), Bc = be - mean*A   [1,512]
            c1 = 1.0 / (B * HW4)
            mean = s1s.tile([1, 512], f32, tag="v1")
            nc.vector.tensor_scalar_mul(mean[:], gath1[:, :512], c1)
            ex2 = s1s.tile([1, 512], f32, tag="v2")
            nc.vector.tensor_scalar_mul(ex2[:], gath1[:, 512:], c1)
            var = s1s.tile([1, 512], f32, tag="v3")
            nc.vector.tensor_mul(var[:], mean[:], mean[:])
            nc.vector.tensor_sub(var[:], ex2[:], var[:])
            if d.get("DEBUG"):
                nc.sync.dma_start(d["dbg_var"][:], var[:])
            std = s1s.tile([1, 512], f32, tag="v4")
            nc.scalar.activation(std[:], var[:], AF.Sqrt, bias=eps1[:])
            rstd = s1s.tile([1, 512], f32, tag="v5")
            nc.vector.reciprocal(rstd[:], std[:])
            Ar = s1s.tile([1, 512], f32r, tag="v6")
            nc.vector.tensor_mul(Ar[:], bn4gb[:, :512], rstd[:])
            mA = s1s.tile([1, 512], f32, tag="v7")
            nc.vector.tensor_mul(mA[:], mean[:], Ar[:])
            Br = s1s.tile([1, 512], f32r, tag="v8")
            nc.vector.tensor_sub(Br[:], bn4gb[:, 512:], mA[:])
            bA = tmp_ps(121, 512)
            nc.tensor.matmul(bA[:], ones_sb[:1, :121], Ar[:], start=True, stop=True)
            bB = tmp_ps(121, 512)
            nc.tensor.matmul(bB[:], ones_sb[:1, :121], Br[:], start=True, stop=True)
            t1 = s1p.tile([121, 512], f32, tag="zt1", name="t1")
            nc.vector.tensor_mul(t1[:], zqkT[:], bA[:])
            nc.vector.tensor_add(t1[:], t1[:], bB[:])
            # relu on DVE (max with 0), bf16 out; zero rows 121..127 so PE
            # transposes have even input
            r4qkT = s1p.tile([128, 512], f32)
            nc.vector.memset(r4qkT[:], 0.0)
            nc.vector.tensor_scalar_max(r4qkT[:121, :], t1[:], 0.0)

            if d.get("DEBUG"):
                nc.sync.dma_start(d["dbg_r4qkT"][:], r4qkT[:].bitcast(f32))
            # att43 logits: [121, 484]
            r4q = s1p.tile([128, 2 * HW4], bf16)
            for ct in range(2):
                trp = tmp_ps(128, 128)
                nc.tensor.transpose(
                    trp[:], r4qkT[:, ct * 128:(ct + 1) * 128], ident[:])
                nc.vector.tensor_copy(r4q[:, ct * HW4:(ct + 1) * HW4],
                                      trp[:, :HW4])
            l43 = tmp_ps(121, HW3)
            for ct in range(2):
                nc.tensor.matmul(l43[:], r4q[:, ct * HW4:(ct + 1) * HW4],
                                 r3[:, ct * HW3:(ct + 1) * HW3],
                                 start=(ct == 0), stop=(ct == 1))
            att43 = s1p.tile([121, HW3], bf16)
            s43 = s1s.tile([121, 1], f32, tag="s43")
            nc.scalar.activation(att43[:], l43[:], AF.Exp, accum_out=s43[:])
            if d.get("DEBUG"):
                nc.sync.dma_start(d["dbg_s43"][:], s43[:])
            rec43 = s1s.tile([121, 1], f32, tag="r43")
            nc.vector.reciprocal(rec43[:], s43[:])
            r4kTs = s1p.tile([121, 256], bf16)
            nc.vector.tensor_scalar_mul(r4kTs[:], r4qkT[:121, 256:512],
                                        rec43[:])

            # s3 = r34 + r3 -> s3pad (bf16), then q3/k3 conv1d(k=3)
            s3pad = s1p.tile([128, 2 * 486], bf16)
            for ct in range(2):
                nc.vector.memset(s3pad[:, ct * 486: ct * 486 + 1], 0.0)
                nc.vector.memset(s3pad[:, ct * 486 + 485: ct * 486 + 486], 0.0)
            for ct in range(2):
                r34 = tmp_ps(128, HW3)
                nc.tensor.matmul(r34[:], r4kTs[:, ct * 128:(ct + 1) * 128],
                                 att43[:], start=True, stop=True)
                nc.vector.tensor_add(
                    s3pad[:, ct * 486 + 1: ct * 486 + 485], r34[:],
                    r3[:, ct * HW3:(ct + 1) * HW3])
            k3 = s1p.tile([128, HW3], f32)
            for qk in range(2):
                pq = tmp_ps(128, HW3)
                for kt in range(2):
                    for tap in range(3):
                        nc.tensor.matmul(
                            pq[:], w13[:, ((kt * 2 + qk) * 3 + tap) * 128:
                                        ((kt * 2 + qk) * 3 + tap + 1) * 128],
                            s3pad[:, kt * 486 + tap: kt * 486 + tap + HW3],
                            start=(kt == 0 and tap == 0),
                            stop=(kt == 1 and tap == 2))
                if qk == 0:
                    nc.scalar.activation(q3[:], pq[:], AF.Identity,
                                         bias=vecs[:, 3:4])
                else:
                    nc.scalar.activation(k3[:], pq[:], AF.Identity,
                                         bias=vecs[:, 4:5])
            for mt in range(4):
                cw = 128 if mt < 3 else 100
                trp = tmp_ps(cw, 128)
                nc.tensor.transpose(trp[:], k3[:, mt * 128: mt * 128 + cw],
                                    ident[:])
                nc.vector.tensor_copy(k3T[:cw, mt * 128:(mt + 1) * 128], trp[:])

        if d.get("DEBUG"):
            nc.sync.dma_start(d["dbg_q3"][:], q3[:].bitcast(f32))
        # ============ PHASE B: att32 -> r23 -> s2 ============
        r23 = ps_big.tile([128, 2048], f32, tag="big")
        MT3 = [(0, 128), (128, 128), (256, 128), (384, 100)]
        for mi, (q0, mp) in enumerate(MT3):
            att = attp.tile([128, HW2], bf16, tag="att")
            ssum = scr.tile([128, 2], f32, tag="ssum")
            for half in range(2):
                lg = tmp_ps(128, 1024)
                for nb in range(2):
                    col = half * 968 + nb * 484
                    nc.tensor.matmul(lg[:mp, nb * 512: nb * 512 + 484],
                                     q3[:, q0: q0 + mp], r2[:, col: col + 484],
                                     start=True, stop=True)
                lgv = lg[:].rearrange("p (b c) -> p b c", c=512)[:mp, :, :484]
                nc.scalar.activation(att[:mp, half * 968:(half + 1) * 968],
                                     lgv, AF.Exp,
                                     accum_out=ssum[:mp, half: half + 1])
            s32 = scr.tile([128, 1], f32, tag="s32")
            nc.vector.tensor_add(s32[:mp], ssum[:mp, 0:1], ssum[:mp, 1:2])
            rec = scr.tile([128, 1], f32, tag="rec32")
            nc.vector.reciprocal(rec[:mp], s32[:mp])
            kTs = scr.tile([128, 128], bf16, tag="k3Ts")
            nc.vector.tensor_scalar_mul(kTs[:mp], k3T[:mp, q0: q0 + 128],
                                        rec[:mp])
            for nb in range(4):
                nc.tensor.matmul(r23[:, nb * 512: nb * 512 + 484], kTs[:mp],
                                 att[:mp, nb * 484:(nb + 1) * 484],
                                 start=(mi == 0), stop=(mi == 3))
        for b4 in range(4):
            nc.vector.tensor_add(
                s2pad[:, 1 + b4 * 484: 1 + (b4 + 1) * 484],
                r23[:, b4 * 512: b4 * 512 + 484],
                r2[:, b4 * 484:(b4 + 1) * 484])

        if d.get("DEBUG"):
            nc.sync.dma_start(d["dbg_s2"][:], s2pad[:].bitcast(f32))
        # ============ PHASE C: q2/k2 conv1d(k=5) + k2 transposes ============
        CH1 = [(0, 484), (484, 484), (968, 484), (1452, 482)]
        for qk in range(2):
            for st, sz in CH1:
                pq = tmp_ps(128, sz)
                for tap in range(5):
                    nc.tensor.matmul(
                        pq[:], w12[:, (qk * 5 + tap) * 128:
                                    (qk * 5 + tap + 1) * 128],
                        s2pad[:, st + tap: st + tap + sz],
                        start=(tap == 0), stop=(tap == 4))
                if qk == 0:
                    nc.vector.tensor_scalar_add(q2[:, st: st + sz], pq[:],
                                                vecs[:, 5:6])
                else:
                    nc.vector.tensor_scalar_add(k2[:, st: st + sz], pq[:],
                                                vecs[:, 6:7])
        for mt in range(16):
            cw = 128 if mt < 15 else 14
            trp = tmp_ps(cw, 128)
            nc.tensor.transpose(trp[:], k2[:, mt * 128: mt * 128 + cw], ident[:])
            nc.vector.tensor_copy(k2T[:cw, mt * 128:(mt + 1) * 128], trp[:])

        if d.get("DEBUG"):
            nc.sync.dma_start(d["dbg_q2"][:], q2[:].bitcast(f32))
            nc.sync.dma_start(d["dbg_k2T"][:], k2T[:].bitcast(f32))
        # keep the CC engine warm so the BN2 collective starts fast;
        # gated on a phase-C result so it fires shortly before phase E
        nc.sync.dma_start(d["cc0_in"][:], k2T[0:1, 0:16].bitcast(f32r))
        nc.gpsimd.collective_compute(
            "AllReduce", ALU.add, replica_groups=d["RG"],
            ins=[d["cc0_in"][:].opt()], outs=[d["cc0c_out"][:].opt()])

        # ============ PHASE D: att22 -> r2f ============
        r2f = ps_big.tile([128, 2048], f32, tag="big")
        for mt in range(16):
            mp = 128 if mt < 15 else 14
            q0 = mt * 128
            att = attp.tile([128, HW2], bf16, tag="att")
            ssum = scr.tile([128, 2], f32, tag="ssum")
            for half in range(2):
                lg = tmp_ps(128, 1024)
                for nb in range(2):
                    col = half * 968 + nb * 484
                    nc.tensor.matmul(lg[:mp, nb * 512: nb * 512 + 484],
                                     q2[:, q0: q0 + mp], r2[:, col: col + 484],
                                     start=True, stop=True)
                lgv = lg[:].rearrange("p (b c) -> p b c", c=512)[:mp, :, :484]
                nc.scalar.activation(att[:mp, half * 968:(half + 1) * 968],
                                     lgv, AF.Exp,
                                     accum_out=ssum[:mp, half: half + 1])
            s22 = scr.tile([128, 1], f32, tag="s32")
            nc.vector.tensor_add(s22[:mp], ssum[:mp, 0:1], ssum[:mp, 1:2])
            rec = scr.tile([128, 1], f32, tag="rec32")
            nc.vector.reciprocal(rec[:mp], s22[:mp])
            kTs = scr.tile([128, 128], bf16, tag="k3Ts")
            nc.vector.tensor_scalar_mul(kTs[:mp], k2T[:mp, q0: q0 + 128],
                                        rec[:mp])
            for nb in range(4):
                nc.tensor.matmul(r2f[:, nb * 512: nb * 512 + 484], kTs[:mp],
                                 att[:mp, nb * 484:(nb + 1) * 484],
                                 start=(mt == 0), stop=(mt == 15))
        # r2f -> padded grid interior (4 bank-strided pieces of 11 rows)
        for b4 in range(4):
            nc.vector.tensor_copy(
                _view2d(r2fpad[:, :PG2], 46)[:, 1 + 11 * b4: 12 + 11 * b4, 1:45],
                _view2d(r2f[:, b4 * 512: b4 * 512 + 484], 44))

        # ============ PHASE E: final conv + BN2 + residual ============
        ysums = scr.tile([128, 4], f32, tag="ysums")
        ysqs = scr.tile([128, 4], f32, tag="ysqs")
        for ci, (st, sz) in enumerate(CH2):
            pc = tmp_ps(128, sz)
            for tap in range(9):
                dy, dx = tap // 3, tap % 3
                nc.tensor.matmul(
                    pc[:], w2r[:, tap * 128:(tap + 1) * 128],
                    r2fpad[:, st + dy * 46 + dx: st + dy * 46 + dx + sz],
                    start=(tap == 0), stop=(tap == 8))
            r0 = st // 46
            src = _view2d(pc[:, :11 * 46], 46)[:, :, :44]
            nc.scalar.activation(
                y_sb[:, r0 * 44: r0 * 44 + 11 * 44], src,
                AF.Identity, accum_out=ysums[:, ci: ci + 1])
            ysq_s = scr.tile([128, 506], bf16, tag="ysq_s")
            nc.scalar.activation(ysq_s[:, :11 * 44], src, AF.Square,
                                 accum_out=ysqs[:, ci: ci + 1])
        if d.get("DEBUG"):
            nc.sync.dma_start(d["dbg_y"][:], y_sb[:])
        ysum_f = acts.tile([128, 2], f32)
        nc.vector.reduce_sum(ysum_f[:, 0:1], ysums[:], axis=AX.X)
        nc.vector.reduce_sum(ysum_f[:, 1:2], ysqs[:], axis=AX.X)
        nc.sync.dma_start(d["cc2_in"][:], ysum_f[:].bitcast(f32r))
        nc.gpsimd.collective_compute(
            "AllReduce", ALU.add, replica_groups=d["RG"],
            ins=[d["cc2_in"][:].opt()], outs=[d["cc2_out"][:].opt()])
        gs2 = acts.tile([128, 2], f32r)
        nc.sync.dma_start(gs2[:], d["cc2_out"][:])
        if d.get("DEBUG"):
            nc.sync.dma_start(d["dbg_gs2"][:], gs2[:].bitcast(f32))
        c2c = 1.0 / (B * HW2)
        mean2 = acts.tile([128, 1], f32)
        nc.vector.tensor_scalar_mul(mean2[:], gs2[:, 0:1], c2c)
        var2 = acts.tile([128, 1], f32)
        nc.vector.tensor_mul(var2[:], mean2[:], mean2[:])
        # var2 = ex2*c2c - mean2^2 + eps  (fold eps in here)
        nc.vector.scalar_tensor_tensor(var2[:], gs2[:, 1:2], c2c, var2[:],
                                       ALU.mult, ALU.subtract)
        nc.vector.tensor_scalar_add(var2[:], var2[:], EPS)
        # rstd2 = rsqrt(var2) via bit-magic seed + 2 Newton iterations (DVE
        # only — avoids an ACT sqrt table swap on the tail critical path)
        rs = acts.tile([128, 1], f32)
        rsi = rs[:].bitcast(i32)
        # seed = magic - (i >> 1) = (magic + 1) + ~(i >> 1)  (two's complement)
        nc.vector.tensor_scalar(rsi, var2[:].bitcast(i32), 1, -1,
                                ALU.logical_shift_right, ALU.bitwise_xor)
        nc.vector.tensor_scalar(rsi, rsi, 0x5f3759e0, None, ALU.add)
        half_v = acts.tile([128, 1], f32)
        nc.vector.tensor_scalar_mul(half_v[:], var2[:], 0.5)
        tmp_n = acts.tile([128, 1], f32)
        for _ in range(3):
            # x = x * (1.5 - half_v * x * x)
            nc.vector.tensor_mul(tmp_n[:], rs[:], rs[:])
            nc.vector.tensor_mul(tmp_n[:], tmp_n[:], half_v[:])
            nc.vector.tensor_scalar(tmp_n[:], tmp_n[:], -1.0, 1.5,
                                    ALU.mult, ALU.add)
            nc.vector.tensor_mul(rs[:], rs[:], tmp_n[:])
        A2 = acts.tile([128, 1], f32)
        nc.vector.tensor_mul(A2[:], vecs[:, 7:8], rs[:])
        mA2 = acts.tile([128, 1], f32)
        nc.vector.tensor_mul(mA2[:], mean2[:], A2[:])
        B2 = acts.tile([128, 1], f32)
        nc.vector.tensor_sub(B2[:], vecs[:, 8:9], mA2[:])
        if d.get("DEBUG"):
            nc.sync.dma_start(d["dbg_AB"][:, 0:1], A2[:])
            nc.sync.dma_start(d["dbg_AB"][:, 1:2], B2[:])
        out_sb = acts.tile([128, HW2], f32, name="out_sb")
        for hh in range(2):
            cols = slice(hh * 968, (hh + 1) * 968)
            nc.scalar.activation(out_sb[:, cols], y_sb[:, cols], AF.Relu,
                                 bias=B2[:], scale=A2[:])
            ov = _view2d(out_sb[:, cols], 44)
            nc.vector.tensor_add(
                ov, ov,
                _view2d(xp2[:, :PG2], 46)[:, 1 + 22 * hh: 23 + 22 * hh, 1:45])
            nc.sync.dma_start(d["d_out"][:, cols], out_sb[:, cols].bitcast(f32))


# ---------------- host-side input prep ----------------

def _prep_shared(inputs):
    import ml_dtypes
    g = lambda k: np.ascontiguousarray(np.asarray(inputs[k], dtype=np.float32))
    w4qk = np.empty((4, 128, 9, 512), np.float32)
    wq, wk = g("w_r4q"), g("w_r4k")       # [256, 512, 3, 3]
    for t in range(4):
        ci = slice(t * 128, (t + 1) * 128)
        # [128ci, 3,3, 256co] per source
        w4qk[t, :, :, :256] = wq[:, ci].transpose(1, 2, 3, 0).reshape(128, 9, 256)
        w4qk[t, :, :, 256:] = wk[:, ci].transpose(1, 2, 3, 0).reshape(128, 9, 256)
    w4qk = w4qk.reshape(4, 128, 9 * 512).astype(ml_dtypes.bfloat16)

    wn3s = g("w_n3")                      # [256, 256, 3, 3]
    wn3 = np.empty((2, 128, 2, 9, 128), np.float32)
    for kt in range(2):
        for ct in range(2):
            blk = wn3s[ct * 128:(ct + 1) * 128, kt * 128:(kt + 1) * 128]
            wn3[kt, :, ct] = blk.transpose(1, 2, 3, 0).reshape(128, 9, 128)
    wn3 = wn3.reshape(2, 128, 2304)
    wn3 = np.concatenate([wn3[0], wn3[1]], axis=1)      # [128, 4608]

    wn2 = g("w_n2").transpose(1, 2, 3, 0).reshape(128, 1152)
    w2r = g("w_2r").transpose(1, 2, 3, 0).reshape(128, 1152)

    w13 = np.empty((2, 128, 2, 3, 128), np.float32)
    for kt in range(2):
        for qk, key in enumerate(("w1_3q", "w1_3k")):
            blk = g(key)[:, kt * 128:(kt + 1) * 128]    # [128co, 128ci, 3]
            w13[kt, :, qk] = blk.transpose(1, 2, 0)
    w13 = np.concatenate([w13[0].reshape(128, 768),
                          w13[1].reshape(128, 768)], axis=1)

    w12 = np.empty((128, 2, 5, 128), np.float32)
    for qk, key in enumerate(("w1_2q", "w1_2k")):
        w12[:, qk] = g(key).transpose(1, 2, 0)
    w12 = w12.reshape(128, 1280)

    bn4 = np.stack([np.concatenate([g("g_r4q"), g("g_r4k")]),
                    np.concatenate([g("be_r4q"), g("be_r4k")])])
    vecs = np.zeros((128, 10), np.float32)
    vecs[:, 0] = g("b_n3")[:128]
    vecs[:, 1] = g("b_n3")[128:]
    vecs[:, 2] = g("b_n2")
    vecs[:, 3] = g("b1_3q")
    vecs[:, 4] = g("b1_3k")
    vecs[:, 5] = g("b1_2q")
    vecs[:, 6] = g("b1_2k")
    vecs[:, 7] = g("g_2r")
    vecs[:, 8] = g("be_2r")
    ones = np.ones((128, 256), np.float32)
    bf = ml_dtypes.bfloat16
    return dict(w4qk=w4qk, wn3=wn3.astype(bf), wn2=wn2.astype(bf),
                w2r=w2r.astype(bf), w13=w13.astype(bf), w12=w12.astype(bf),
                bn4=bn4, vecs=vecs, onesd=ones)


def _prep_sample(inputs, i):
    import ml_dtypes
    bf = ml_dtypes.bfloat16
    x4 = np.asarray(inputs["x4"][i], dtype=np.float32)   # [512, 11, 11]
    x4p = np.zeros((512, 13, 13), np.float32)
    x4p[:, 1:12, 1:12] = x4
    xp4s = np.empty((4, 128, 9, 121), np.float32)
    for t in range(4):
        ci = slice(t * 128, (t + 1) * 128)
        for tap in range(9):
            dy, dx = tap // 3, tap % 3
            xp4s[t, :, tap] = x4p[ci, dy:dy + 11, dx:dx + 11].reshape(128, 121)
    xp4s = xp4s.reshape(4, 128, 9 * 121).astype(bf)

    x3 = np.asarray(inputs["x3"][i], dtype=np.float32)   # [256, 22, 22]
    x3p = np.zeros((256, 24, 24), np.float32)
    x3p[:, 1:23, 1:23] = x3
    xp3 = np.zeros((2, 128, XP3_W), np.float32)
    xp3[:, :, :PG3] = x3p.reshape(2, 128, PG3)
    xp3 = np.concatenate([xp3[0], xp3[1]], axis=1).astype(bf)  # [128, 2*626]

    x2 = np.asarray(inputs["x2"][i], dtype=np.float32)   # [128, 44, 44]
    x2p = np.zeros((128, 46, 46), np.float32)
    x2p[:, 1:45, 1:45] = x2
    xp2 = np.zeros((128, XP2_W), np.float32)
    xp2[:, :PG2] = x2p.reshape(128, PG2)
    return dict(xp4s=xp4s, xp3=xp3, xp2=xp2, xp2b=xp2.astype(bf))


_NC = None
_NC_KEY = None


def _get_nc(shared, debug=False):
    global _NC, _NC_KEY
    key = sum(int(np.abs(np.asarray(v, dtype=np.float32)).sum() * 997)
              for v in shared.values())
    if _NC is None or _NC_KEY != key:
        _NC = build(shared, DEBUG=debug)
        _NC_KEY = key
    return _NC


def run(inputs, trace=False, debug=False):
    shared = _prep_shared(inputs)
    nc = _get_nc(shared, debug)
    in_maps = [_prep_sample(inputs, i) for i in range(N_CORES)]
    last_err = None
    for attempt in range(3):
        try:
            res = run_bass_kernel_spmd(nc, in_maps, list(range(N_CORES)),
                                       trace=trace)
            break
        except Exception as e:  # transient device errors (NRT_EXEC_UNIT etc.)
            last_err = e
            if attempt == 2:
                raise
    out = np.stack([res.results[i]["out"].reshape(128, 44, 44)
                    for i in range(N_CORES)]).astype(np.float32)
    return out, res


def kernel(**inputs):
    out, _ = run(inputs, trace=False)
    return out


# revision 18
# speedup vs baseline: 1.0640x; 1.0640x over previous
"""Trainium2 Bass kernel for nn_C2M_24378234372461.

Data-parallel over batch (8 samples on 8 NeuronCores). BatchNorm batch
statistics are exchanged with two small collectives whose setup cost is
absorbed by a front-loaded dummy collective. Matmuls run in bf16.
Self-contained: builds + compiles the Bass program on first call.
"""
import sys

for _p in ("/opt/trn_rl_repo",):
    if _p not in sys.path:
        sys.path.append(_p)

import numpy as np
import concourse.bacc as bacc
import concourse.bass as bass
import concourse.mybir as mybir
import concourse.tile as tile
import concourse.masks as masks
from concourse.bass_utils import run_bass_kernel_spmd

f32 = mybir.dt.float32
f32r = mybir.dt.float32r
bf16 = mybir.dt.bfloat16
i32 = mybir.dt.int32
AF = mybir.ActivationFunctionType
AX = mybir.AxisListType
ALU = mybir.AluOpType

N_CORES = 8
B = 8
C2, H2, W2 = 128, 44, 44
C3, H3, W3 = 256, 22, 22
C4, H4, W4 = 512, 11, 11
HW2 = H2 * W2            # 1936
HW3 = H3 * W3            # 484
HW4 = H4 * W4            # 121
PG2 = 46 * 46            # 2116 padded grid scale-2
PG3 = 24 * 24            # 576  padded grid scale-3
XP2_W = PG2 + 96         # flat buffer + tail for overcompute reads (2212)
XP3_W = PG3 + 50         # 626 per cin tile
EPS = 1e-5


def _view2d(ap, width):
    """[p, (rows width)] view of a flat AP."""
    return ap.rearrange("p (r w) -> p r w", w=width)


def build(shared, DEBUG=False):
    nc = bacc.Bacc("TRN2", target_bir_lowering=False, debug=False,
                   num_devices=N_CORES)

    # ---------------- DRAM I/O ----------------
    d_xp4s = nc.dram_tensor("xp4s", [4, 128, 9 * HW4], bf16, kind="ExternalInput")
    d_xp3 = nc.dram_tensor("xp3", [128, 2 * XP3_W], bf16, kind="ExternalInput")
    d_xp2 = nc.dram_tensor("xp2", [128, XP2_W], f32r, kind="ExternalInput")
    d_xp2b = nc.dram_tensor("xp2b", [128, XP2_W], bf16, kind="ExternalInput")
    d_w4qk = nc.inline_tensor(shared["w4qk"], "w4qk")
    d_wn3 = nc.inline_tensor(shared["wn3"], "wn3")
    d_wn2 = nc.inline_tensor(shared["wn2"], "wn2")
    d_w2r = nc.inline_tensor(shared["w2r"], "w2r")
    d_w13 = nc.inline_tensor(shared["w13"], "w13")
    d_w12 = nc.inline_tensor(shared["w12"], "w12")
    d_bn4 = nc.inline_tensor(shared["bn4"], "bn4")
    d_vecs = nc.inline_tensor(shared["vecs"], "vecs")
    d_ones = nc.inline_tensor(shared["onesd"], "onesd")
    d_out = nc.dram_tensor("out", [128, HW2], f32, kind="ExternalOutput")
    if DEBUG:
        dbg_zqkT = nc.dram_tensor("dbg_zqkT", [121, 512], f32, kind="ExternalOutput")
        dbg_gath1 = nc.dram_tensor("dbg_gath1", [8, 1024], f32, kind="ExternalOutput")
        dbg_var = nc.dram_tensor("dbg_var", [1, 512], f32, kind="ExternalOutput")
        dbg_r4qkT = nc.dram_tensor("dbg_r4qkT", [128, 512], f32, kind="ExternalOutput")
        dbg_r3 = nc.dram_tensor("dbg_r3", [128, 968], f32, kind="ExternalOutput")
        dbg_s43 = nc.dram_tensor("dbg_s43", [121, 1], f32, kind="ExternalOutput")
        dbg_q3 = nc.dram_tensor("dbg_q3", [128, 484], f32, kind="ExternalOutput")
        dbg_r2 = nc.dram_tensor("dbg_r2", [128, 1936], f32, kind="ExternalOutput")
        dbg_s2 = nc.dram_tensor("dbg_s2", [128, 1938], f32, kind="ExternalOutput")
        dbg_q2 = nc.dram_tensor("dbg_q2", [128, 1934], f32, kind="ExternalOutput")
        dbg_k2T = nc.dram_tensor("dbg_k2T", [128, 2048], f32, kind="ExternalOutput")
        dbg_y = nc.dram_tensor("dbg_y", [128, 1936], f32, kind="ExternalOutput")
        dbg_gs2 = nc.dram_tensor("dbg_gs2", [128, 2], f32, kind="ExternalOutput")
        dbg_AB = nc.dram_tensor("dbg_AB", [128, 2], f32, kind="ExternalOutput")

    # collective bounce buffers
    cc0_in = nc.dram_tensor("cc0_in", [1, 8], f32r)
    cc0_out = nc.dram_tensor("cc0_out", [1, 8], f32r, addr_space="Shared")
    cc0b_out = nc.dram_tensor("cc0b_out", [1, 8], f32r, addr_space="Shared")
    cc0c_out = nc.dram_tensor("cc0c_out", [1, 8], f32r, addr_space="Shared")
    cc1_in = nc.dram_tensor("cc1_in", [1, 1024], f32r)
    cc1_out = nc.dram_tensor("cc1_out", [1, 1024], f32r, addr_space="Shared")
    cc2_in = nc.dram_tensor("cc2_in", [128, 2], f32r)
    cc2_out = nc.dram_tensor("cc2_out", [128, 2], f32r, addr_space="Shared")
    RG = [list(range(N_CORES))]

    with tile.TileContext(nc) as tc:
        _build_body(nc, tc, locals())
    nc.compile()
    return nc


def _build_body(nc, tc, d):
    from contextlib import ExitStack

    ctx = ExitStack()
    with ctx:
        # Dummy collectives: first ops on the CC queue. The first absorbs
        # the one-time collective setup / rendezvous cost in the background
        # while the tensor engine computes; the second confirms the warm
        # path so the real stats collective behind them is fast.
        nc.gpsimd.collective_compute(
            "AllReduce", ALU.add, replica_groups=d["RG"],
            ins=[d["cc0_in"][:].opt()], outs=[d["cc0_out"][:].opt()])
        const = ctx.enter_context(tc.tile_pool(name="const", bufs=1))
        acts = ctx.enter_context(tc.tile_pool(name="acts", bufs=1))
        scr = ctx.enter_context(tc.tile_pool(name="scr", bufs=3))
        attp = ctx.enter_context(tc.tile_pool(name="attp", bufs=3))
        ps_tmp = ctx.enter_context(tc.tile_pool(name="ps_tmp", bufs=2, space="PSUM"))
        ps_big = ctx.enter_context(tc.tile_pool(name="ps_big", bufs=1, space="PSUM"))

        _tmp_i = [0]
        CH2 = [(0, 506), (506, 506), (1012, 506), (1518, 506)]

        def tmp_ps(p, n):
            _tmp_i[0] += 1
            return ps_tmp.tile([p, n], f32, tag="tmp", name=f"tps{_tmp_i[0]}")

        # ------------- constants / weights (persistent) -------------
        ident = const.tile([128, 128], f32)
        masks.make_identity(nc, ident[:])
        ones_sb = const.tile([128, 256], f32r)
        nc.sync.dma_start(ones_sb[:], d["d_ones"][:].bitcast(f32r))
        vecs = const.tile([128, 10], f32)
        nc.sync.dma_start(vecs[:], d["d_vecs"][:])
        bn4gb = const.tile([1, 1024], f32)
        wn3 = const.tile([128, 4608], bf16)
        wn2 = const.tile([128, 1152], bf16)
        w2r = const.tile([128, 1152], bf16)
        w13 = const.tile([128, 1536], bf16)
        w12 = const.tile([128, 1280], bf16)

        # preload the sqrt activation table during startup so BN1's Sqrt
        # causes no table swap on the critical path
        eps1 = const.tile([1, 1], f32)
        nc.vector.memset(eps1[:], EPS)
        sqd = const.tile([1, 1], f32)
        nc.scalar.activation(sqd[:], eps1[:], AF.Sqrt)

        # ------------- persistent activations -------------
        xp2 = acts.tile([128, XP2_W], f32r)
        xp2b = acts.tile([128, XP2_W], bf16)
        r2 = acts.tile([128, HW2], bf16)
        q3 = acts.tile([128, HW3], bf16)
        k3T = acts.tile([128, 512], bf16)
        s2pad = acts.tile([128, HW2 + 2], bf16)
        q2 = acts.tile([128, 1934], bf16)
        k2 = acts.tile([128, 1934], f32)
        k2T = acts.tile([128, 2048], bf16)
        r2fpad = acts.tile([128, XP2_W], bf16)
        y_sb = acts.tile([128, HW2], f32)

        # zero the padding borders (replaces the zeros DMA)
        nc.vector.memset(s2pad[:, 0:1], 0.0)
        nc.vector.memset(s2pad[:, HW2 + 1: HW2 + 2], 0.0)
        g2 = _view2d(r2fpad[:, :PG2], 46)
        nc.vector.memset(g2[:, 0:1, :], 0.0)
        nc.vector.memset(g2[:, 45:46, :], 0.0)
        nc.vector.memset(g2[:, 1:45, 0:1], 0.0)
        nc.vector.memset(g2[:, 1:45, 45:46], 0.0)
        nc.vector.memset(r2fpad[:, PG2:], 0.0)

        # ============ PHASE A: scale-4 + BN1 + att43 + s3 + q3/k3 ============
        with tc.tile_pool(name="s1", bufs=1) as s1p, \
             tc.tile_pool(name="s1s", bufs=1) as s1s:
            xp3 = s1p.tile([128, 2 * XP3_W], bf16)

            # z^T = conv(x4) for q|k stacked: [121, 512]
            zT = tmp_ps(121, 512)
            with tc.tile_pool(name="s1w", bufs=3) as s1w, \
                 tc.tile_pool(name="s1x", bufs=1) as s1x:
                for t in range(4):
                    xc = s1x.tile([128, 9 * HW4], bf16, tag="x4c",
                                  name=f"x4c{t}")
                    nc.sync.dma_start(xc[:, :545], d["d_xp4s"][t][:, :545])
                    nc.sync.dma_start(xc[:, 545:], d["d_xp4s"][t][:, 545:])
                    for tp2 in range(3):
                        wc = s1w.tile([128, 1536], bf16, tag="w4c",
                                      name=f"w4c{t}_{tp2}")
                        for wsp in range(6):
                            nc.sync.dma_start(
                                wc[:, wsp * 256:(wsp + 1) * 256],
                                d["d_w4qk"][t][:, tp2 * 1536 + wsp * 256:
                                               tp2 * 1536 + (wsp + 1) * 256])
                        for tj in range(3):
                            tap = tp2 * 3 + tj
                            nc.tensor.matmul(
                                zT[:],
                                xc[:, tap * HW4:(tap + 1) * HW4],
                                wc[:, tj * 512:(tj + 1) * 512],
                                start=(t == 0 and tap == 0),
                                stop=(t == 3 and tap == 8))
            # stats -> collective, triggered as early as possible
            zqkT = s1p.tile([121, 512], f32r)
            nc.vector.tensor_copy(zqkT[:], zT[:])
            if d.get("DEBUG"):
                nc.sync.dma_start(d["dbg_zqkT"][:], zqkT[:].bitcast(f32))
            zsq = s1p.tile([121, 512], f32r, tag="zt1", name="zsq")
            nc.vector.tensor_mul(zsq[:], zqkT[:], zqkT[:])
            stats_ps = tmp_ps(1, 1024)
            nc.tensor.matmul(stats_ps[:, :512], ones_sb[:121, :1], zqkT[:],
                             start=True, stop=True)
            nc.tensor.matmul(stats_ps[:, 512:], ones_sb[:121, :1], zsq[:],
                             start=True, stop=True)
            stats1 = s1p.tile([1, 1024], f32r)
            nc.vector.tensor_copy(stats1[:], stats_ps[:])
            nc.sync.dma_start(d["cc1_in"][:], stats1[:])
            nc.gpsimd.collective_compute(
                "AllReduce", ALU.add, replica_groups=d["RG"],
                ins=[d["cc1_in"][:].opt()], outs=[d["cc1_out"][:].opt()])
            gath1 = s1p.tile([1, 1024], f32r)
            nc.sync.dma_start(gath1[:], d["cc1_out"][:])
            # PE warmup burst (HAM unthrottle) bridges startup DMA waits
            for wi in range(8):
                wp = tmp_ps(128, 256)
                nc.tensor.matmul(wp[:], ones_sb[:, :128], ones_sb[:],
                                 start=True, stop=True)
            # deferred input DMAs (after the stage-1 critical path)
            nc.sync.dma_start(xp3[:, :XP3_W], d["d_xp3"][:, :XP3_W])
            nc.sync.dma_start(xp3[:, XP3_W:], d["d_xp3"][:, XP3_W:])
            for wsp in range(4):
                nc.sync.dma_start(wn3[:, wsp * 1152:(wsp + 1) * 1152],
                                  d["d_wn3"][:, wsp * 1152:(wsp + 1) * 1152])
            nc.sync.dma_start(xp2b[:, :1106], d["d_xp2b"][:, :1106])
            nc.sync.dma_start(xp2b[:, 1106:], d["d_xp2b"][:, 1106:])
            nc.sync.dma_start(wn2[:], d["d_wn2"][:])
            nc.sync.dma_start(bn4gb[:, :512], d["d_bn4"][0:1, :])
            nc.sync.dma_start(bn4gb[:, 512:], d["d_bn4"][1:2, :])
            nc.sync.dma_start(w13[:], d["d_w13"][:])
            nc.sync.dma_start(w12[:], d["d_w12"][:])
            nc.sync.dma_start(w2r[:], d["d_w2r"][:])
            nc.sync.dma_start(xp2[:, :1106], d["d_xp2"][:, :1106])
            nc.sync.dma_start(xp2[:, 1106:], d["d_xp2"][:, 1106:])

            # r3 = conv_n3(x3) + bias  [2ct][128, 484] — independent of the
            # collective; fills its latency window on the tensor queue
            r3 = s1p.tile([128, 2 * HW3], bf16)
            for ct in range(2):
                for ch in range(2):          # padded-grid chunks of 288 (12 rows)
                    pc = tmp_ps(128, 288)
                    for kt in range(2):
                        for tap in range(9):
                            dy, dx = tap // 3, tap % 3
                            off = kt * XP3_W + ch * 288 + dy * 24 + dx
                            nc.tensor.matmul(
                                pc[:], wn3[:, (kt * 18 + ct * 9 + tap) * 128:
                                           (kt * 18 + ct * 9 + tap + 1) * 128],
                                xp3[:, off: off + 288],
                                start=(kt == 0 and tap == 0),
                                stop=(kt == 1 and tap == 8))
                    r0 = ch * 12
                    nr = min(12, 22 - r0)
                    src = _view2d(pc[:, :nr * 24], 24)[:, :, :22]
                    nc.scalar.activation(
                        r3[:, ct * HW3 + r0 * 22:
                           ct * HW3 + r0 * 22 + nr * 22],
                        src, AF.Identity, bias=vecs[:, ct:ct + 1])
            if d.get("DEBUG"):
                nc.sync.dma_start(d["dbg_r3"][:], r3[:].bitcast(f32))

            # r2 conv (also collective-independent)
            for ci, (st, sz) in enumerate(CH2):
                pc = tmp_ps(128, sz)
                for tap in range(9):
                    dy, dx = tap // 3, tap % 3
                    nc.tensor.matmul(
                        pc[:], wn2[:, tap * 128:(tap + 1) * 128],
                        xp2b[:, st + dy * 46 + dx: st + dy * 46 + dx + sz],
                        start=(tap == 0), stop=(tap == 8))
                r0 = st // 46
                src = _view2d(pc[:, :11 * 46], 46)[:, :, :44]
                nc.scalar.activation(
                    r2[:, r0 * 44: r0 * 44 + 11 * 44], src,
                    AF.Identity, bias=vecs[:, 2:3])
            if d.get("DEBUG"):
                nc.sync.dma_start(d["dbg_r2"][:], r2[:].bitcast(f32))

            # keep-warm fillers bridge the BN1 collective stall so the
            # post-BN1 phases start at full PE clock
            for wi in range(48):
                wp = tmp_ps(128, 256)
                nc.tensor.matmul(wp[:], ones_sb[:, :128], ones_sb[:],
                                 start=True, stop=True)
            # BN affine: A = g * rsqrt(var+eps), Bc = be - mean*A   [1,512]
            c1 = 1.0 / (B * HW4)
            mean = s1s.tile([1, 512], f32, tag="v1")
            nc.vector.tensor_scalar_mul(mean[:], gath1[:, :512], c1)
            ex2 = s1s.tile([1, 512], f32, tag="v2")
            nc.vector.tensor_scalar_mul(ex2[:], gath1[:, 512:], c1)
            var = s1s.tile([1, 512], f32, tag="v3")
            nc.vector.tensor_mul(var[:], mean[:], mean[:])
            nc.vector.tensor_sub(var[:], ex2[:], var[:])
            if d.get("DEBUG"):
                nc.sync.dma_start(d["dbg_var"][:], var[:])
            std = s1s.tile([1, 512], f32, tag="v4")
            nc.scalar.activation(std[:], var[:], AF.Sqrt, bias=eps1[:])
            rstd = s1s.tile([1, 512], f32, tag="v5")
            nc.vector.reciprocal(rstd[:], std[:])
            Ar = s1s.tile([1, 512], f32r, tag="v6")
            nc.vector.tensor_mul(Ar[:], bn4gb[:, :512], rstd[:])
            mA = s1s.tile([1, 512], f32, tag="v7")
            nc.vector.tensor_mul(mA[:], mean[:], Ar[:])
            Br = s1s.tile([1, 512], f32r, tag="v8")
            nc.vector.tensor_sub(Br[:], bn4gb[:, 512:], mA[:])
            bA = tmp_ps(121, 512)
            nc.tensor.matmul(bA[:], ones_sb[:1, :121], Ar[:], start=True, stop=True)
            bB = tmp_ps(121, 512)
            nc.tensor.matmul(bB[:], ones_sb[:1, :121], Br[:], start=True, stop=True)
            t1 = s1p.tile([121, 512], f32, tag="zt1", name="t1")
            nc.vector.tensor_mul(t1[:], zqkT[:], bA[:])
            nc.vector.tensor_add(t1[:], t1[:], bB[:])
            # relu on DVE (max with 0), bf16 out; zero rows 121..127 so PE
            # transposes have even input
            r4qkT = s1p.tile([128, 512], f32)
            nc.vector.memset(r4qkT[:], 0.0)
            nc.vector.tensor_scalar_max(r4qkT[:121, :], t1[:], 0.0)

            if d.get("DEBUG"):
                nc.sync.dma_start(d["dbg_r4qkT"][:], r4qkT[:].bitcast(f32))
            # att43 logits: [121, 484]
            r4q = s1p.tile([128, 2 * HW4], bf16)
            for ct in range(2):
                trp = tmp_ps(128, 128)
                nc.tensor.transpose(
                    trp[:], r4qkT[:, ct * 128:(ct + 1) * 128], ident[:])
                nc.vector.tensor_copy(r4q[:, ct * HW4:(ct + 1) * HW4],
                                      trp[:, :HW4])
            l43 = tmp_ps(121, HW3)
            for ct in range(2):
                nc.tensor.matmul(l43[:], r4q[:, ct * HW4:(ct + 1) * HW4],
                                 r3[:, ct * HW3:(ct + 1) * HW3],
                                 start=(ct == 0), stop=(ct == 1))
            att43 = s1p.tile([121, HW3], bf16)
            s43 = s1s.tile([121, 1], f32, tag="s43")
            nc.scalar.activation(att43[:], l43[:], AF.Exp, accum_out=s43[:])
            if d.get("DEBUG"):
                nc.sync.dma_start(d["dbg_s43"][:], s43[:])
            rec43 = s1s.tile([121, 1], f32, tag="r43")
            nc.vector.reciprocal(rec43[:], s43[:])
            r4kTs = s1p.tile([121, 256], bf16)
            nc.vector.tensor_scalar_mul(r4kTs[:], r4qkT[:121, 256:512],
                                        rec43[:])

            # s3 = r34 + r3 -> s3pad (bf16), then q3/k3 conv1d(k=3)
            s3pad = s1p.tile([128, 2 * 486], bf16)
            for ct in range(2):
                nc.vector.memset(s3pad[:, ct * 486: ct * 486 + 1], 0.0)
                nc.vector.memset(s3pad[:, ct * 486 + 485: ct * 486 + 486], 0.0)
            for ct in range(2):
                r34 = tmp_ps(128, HW3)
                nc.tensor.matmul(r34[:], r4kTs[:, ct * 128:(ct + 1) * 128],
                                 att43[:], start=True, stop=True)
                nc.vector.tensor_add(
                    s3pad[:, ct * 486 + 1: ct * 486 + 485], r34[:],
                    r3[:, ct * HW3:(ct + 1) * HW3])
            k3 = s1p.tile([128, HW3], f32)
            for qk in range(2):
                pq = tmp_ps(128, HW3)
                for kt in range(2):
                    for tap in range(3):
                        nc.tensor.matmul(
                            pq[:], w13[:, ((kt * 2 + qk) * 3 + tap) * 128:
                                        ((kt * 2 + qk) * 3 + tap + 1) * 128],
                            s3pad[:, kt * 486 + tap: kt * 486 + tap + HW3],
                            start=(kt == 0 and tap == 0),
                            stop=(kt == 1 and tap == 2))
                if qk == 0:
                    nc.scalar.activation(q3[:], pq[:], AF.Identity,
                                         bias=vecs[:, 3:4])
                else:
                    nc.scalar.activation(k3[:], pq[:], AF.Identity,
                                         bias=vecs[:, 4:5])
            for mt in range(4):
                cw = 128 if mt < 3 else 100
                trp = tmp_ps(cw, 128)
                nc.tensor.transpose(trp[:], k3[:, mt * 128: mt * 128 + cw],
                                    ident[:])
                nc.vector.tensor_copy(k3T[:cw, mt * 128:(mt + 1) * 128], trp[:])

        if d.get("DEBUG"):
            nc.sync.dma_start(d["dbg_q3"][:], q3[:].bitcast(f32))
        # ============ PHASE B: att32 -> r23 -> s2 ============
        r23 = ps_big.tile([128, 2048], f32, tag="big")
        MT3 = [(0, 128), (128, 128), (256, 128), (384, 100)]
        for mi, (q0, mp) in enumerate(MT3):
            att = attp.tile([128, HW2], bf16, tag="att")
            ssum = scr.tile([128, 2], f32, tag="ssum")
            for half in range(2):
                lg = tmp_ps(128, 1024)
                for nb in range(2):
                    col = half * 968 + nb * 484
                    nc.tensor.matmul(lg[:mp, nb * 512: nb * 512 + 484],
                                     q3[:, q0: q0 + mp], r2[:, col: col + 484],
                                     start=True, stop=True)
                lgv = lg[:].rearrange("p (b c) -> p b c", c=512)[:mp, :, :484]
                nc.scalar.activation(att[:mp, half * 968:(half + 1) * 968],
                                     lgv, AF.Exp,
                                     accum_out=ssum[:mp, half: half + 1])
            s32 = scr.tile([128, 1], f32, tag="s32")
            nc.vector.tensor_add(s32[:mp], ssum[:mp, 0:1], ssum[:mp, 1:2])
            rec = scr.tile([128, 1], f32, tag="rec32")
            nc.vector.reciprocal(rec[:mp], s32[:mp])
            kTs = scr.tile([128, 128], bf16, tag="k3Ts")
            nc.vector.tensor_scalar_mul(kTs[:mp], k3T[:mp, q0: q0 + 128],
                                        rec[:mp])
            for nb in range(4):
                nc.tensor.matmul(r23[:, nb * 512: nb * 512 + 484], kTs[:mp],
                                 att[:mp, nb * 484:(nb + 1) * 484],
                                 start=(mi == 0), stop=(mi == 3))
        for b4 in range(4):
            nc.vector.tensor_add(
                s2pad[:, 1 + b4 * 484: 1 + (b4 + 1) * 484],
                r23[:, b4 * 512: b4 * 512 + 484],
                r2[:, b4 * 484:(b4 + 1) * 484])

        if d.get("DEBUG"):
            nc.sync.dma_start(d["dbg_s2"][:], s2pad[:].bitcast(f32))
        # ============ PHASE C: q2/k2 conv1d(k=5) + k2 transposes ============
        CH1 = [(0, 484), (484, 484), (968, 484), (1452, 482)]
        for qk in range(2):
            for st, sz in CH1:
                pq = tmp_ps(128, sz)
                for tap in range(5):
                    nc.tensor.matmul(
                        pq[:], w12[:, (qk * 5 + tap) * 128:
                                    (qk * 5 + tap + 1) * 128],
                        s2pad[:, st + tap: st + tap + sz],
                        start=(tap == 0), stop=(tap == 4))
                if qk == 0:
                    nc.vector.tensor_scalar_add(q2[:, st: st + sz], pq[:],
                                                vecs[:, 5:6])
                else:
                    nc.vector.tensor_scalar_add(k2[:, st: st + sz], pq[:],
                                                vecs[:, 6:7])
        for mt in range(16):
            cw = 128 if mt < 15 else 14
            trp = tmp_ps(cw, 128)
            nc.tensor.transpose(trp[:], k2[:, mt * 128: mt * 128 + cw], ident[:])
            nc.vector.tensor_copy(k2T[:cw, mt * 128:(mt + 1) * 128], trp[:])

        if d.get("DEBUG"):
            nc.sync.dma_start(d["dbg_q2"][:], q2[:].bitcast(f32))
            nc.sync.dma_start(d["dbg_k2T"][:], k2T[:].bitcast(f32))
        # mid-kernel warm CC (background; keeps the CC engine streak alive)
        nc.gpsimd.collective_compute(
            "AllReduce", ALU.add, replica_groups=d["RG"],
            ins=[d["cc0_in"][:].opt()], outs=[d["cc0c_out"][:].opt()])

        # ============ PHASE D: att22 -> r2f ============
        r2f = ps_big.tile([128, 2048], f32, tag="big")
        for mt in range(16):
            mp = 128 if mt < 15 else 14
            q0 = mt * 128
            att = attp.tile([128, HW2], bf16, tag="att")
            ssum = scr.tile([128, 2], f32, tag="ssum")
            for half in range(2):
                lg = tmp_ps(128, 1024)
                for nb in range(2):
                    col = half * 968 + nb * 484
                    nc.tensor.matmul(lg[:mp, nb * 512: nb * 512 + 484],
                                     q2[:, q0: q0 + mp], r2[:, col: col + 484],
                                     start=True, stop=True)
                lgv = lg[:].rearrange("p (b c) -> p b c", c=512)[:mp, :, :484]
                nc.scalar.activation(att[:mp, half * 968:(half + 1) * 968],
                                     lgv, AF.Exp,
                                     accum_out=ssum[:mp, half: half + 1])
            s22 = scr.tile([128, 1], f32, tag="s32")
            nc.vector.tensor_add(s22[:mp], ssum[:mp, 0:1], ssum[:mp, 1:2])
            rec = scr.tile([128, 1], f32, tag="rec32")
            nc.vector.reciprocal(rec[:mp], s22[:mp])
            kTs = scr.tile([128, 128], bf16, tag="k3Ts")
            nc.vector.tensor_scalar_mul(kTs[:mp], k2T[:mp, q0: q0 + 128],
                                        rec[:mp])
            for nb in range(4):
                nc.tensor.matmul(r2f[:, nb * 512: nb * 512 + 484], kTs[:mp],
                                 att[:mp, nb * 484:(nb + 1) * 484],
                                 start=(mt == 0), stop=(mt == 15))
        # r2f -> padded grid interior (4 bank-strided pieces of 11 rows)
        for b4 in range(4):
            nc.vector.tensor_copy(
                _view2d(r2fpad[:, :PG2], 46)[:, 1 + 11 * b4: 12 + 11 * b4, 1:45],
                _view2d(r2f[:, b4 * 512: b4 * 512 + 484], 44))

        # warm gate rides the last r2fpad interior write (b4=3 copy)
        nc.sync.dma_start(d["cc0_in"][:],
                          r2fpad[0:1, 1566:1582].bitcast(f32r))
        nc.gpsimd.collective_compute(
            "AllReduce", ALU.add, replica_groups=d["RG"],
            ins=[d["cc0_in"][:].opt()], outs=[d["cc0c_out"][:].opt()])
        # ============ PHASE E: final conv + BN2 + residual ============
        ysums = scr.tile([128, 4], f32, tag="ysums")
        ysqs = scr.tile([128, 4], f32, tag="ysqs")
        for ci, (st, sz) in enumerate(CH2):
            pc = tmp_ps(128, sz)
            for tap in range(9):
                dy, dx = tap // 3, tap % 3
                nc.tensor.matmul(
                    pc[:], w2r[:, tap * 128:(tap + 1) * 128],
                    r2fpad[:, st + dy * 46 + dx: st + dy * 46 + dx + sz],
                    start=(tap == 0), stop=(tap == 8))
            r0 = st // 46
            src = _view2d(pc[:, :11 * 46], 46)[:, :, :44]
            nc.scalar.activation(
                y_sb[:, r0 * 44: r0 * 44 + 11 * 44], src,
                AF.Identity, accum_out=ysums[:, ci: ci + 1])
            ysq_s = scr.tile([128, 506], bf16, tag="ysq_s")
            nc.scalar.activation(ysq_s[:, :11 * 44], src, AF.Square,
                                 accum_out=ysqs[:, ci: ci + 1])
        if d.get("DEBUG"):
            nc.sync.dma_start(d["dbg_y"][:], y_sb[:])
        ysum_f = acts.tile([128, 2], f32)
        nc.vector.reduce_sum(ysum_f[:, 0:1], ysums[:], axis=AX.X)
        nc.vector.reduce_sum(ysum_f[:, 1:2], ysqs[:], axis=AX.X)
        nc.sync.dma_start(d["cc2_in"][:], ysum_f[:].bitcast(f32r))
        nc.gpsimd.collective_compute(
            "AllReduce", ALU.add, replica_groups=d["RG"],
            ins=[d["cc2_in"][:].opt()], outs=[d["cc2_out"][:].opt()])
        gs2 = acts.tile([128, 2], f32r)
        nc.sync.dma_start(gs2[:], d["cc2_out"][:])
        if d.get("DEBUG"):
            nc.sync.dma_start(d["dbg_gs2"][:], gs2[:].bitcast(f32))
        c2c = 1.0 / (B * HW2)
        mean2 = acts.tile([128, 1], f32)
        nc.vector.tensor_scalar_mul(mean2[:], gs2[:, 0:1], c2c)
        var2 = acts.tile([128, 1], f32)
        nc.vector.tensor_mul(var2[:], mean2[:], mean2[:])
        # var2 = ex2*c2c - mean2^2 + eps  (fold eps in here)
        nc.vector.scalar_tensor_tensor(var2[:], gs2[:, 1:2], c2c, var2[:],
                                       ALU.mult, ALU.subtract)
        nc.vector.tensor_scalar_add(var2[:], var2[:], EPS)
        # rstd2 = rsqrt(var2) via bit-magic seed + 2 Newton iterations (DVE
        # only — avoids an ACT sqrt table swap on the tail critical path)
        rs = acts.tile([128, 1], f32)
        rsi = rs[:].bitcast(i32)
        # seed = magic - (i >> 1) = (magic + 1) + ~(i >> 1)  (two's complement)
        nc.vector.tensor_scalar(rsi, var2[:].bitcast(i32), 1, -1,
                                ALU.logical_shift_right, ALU.bitwise_xor)
        nc.vector.tensor_scalar(rsi, rsi, 0x5f3759e0, None, ALU.add)
        half_v = acts.tile([128, 1], f32)
        nc.vector.tensor_scalar_mul(half_v[:], var2[:], 0.5)
        tmp_n = acts.tile([128, 1], f32)
        for _ in range(3):
            # x = x * (1.5 - half_v * x * x)
            nc.vector.tensor_mul(tmp_n[:], rs[:], rs[:])
            nc.vector.tensor_mul(tmp_n[:], tmp_n[:], half_v[:])
            nc.vector.tensor_scalar(tmp_n[:], tmp_n[:], -1.0, 1.5,
                                    ALU.mult, ALU.add)
            nc.vector.tensor_mul(rs[:], rs[:], tmp_n[:])
        A2 = acts.tile([128, 1], f32)
        nc.vector.tensor_mul(A2[:], vecs[:, 7:8], rs[:])
        mA2 = acts.tile([128, 1], f32)
        nc.vector.tensor_mul(mA2[:], mean2[:], A2[:])
        B2 = acts.tile([128, 1], f32)
        nc.vector.tensor_sub(B2[:], vecs[:, 8:9], mA2[:])
        if d.get("DEBUG"):
            nc.sync.dma_start(d["dbg_AB"][:, 0:1], A2[:])
            nc.sync.dma_start(d["dbg_AB"][:, 1:2], B2[:])
        out_sb = acts.tile([128, HW2], f32, name="out_sb")
        for hh in range(2):
            cols = slice(hh * 968, (hh + 1) * 968)
            nc.scalar.activation(out_sb[:, cols], y_sb[:, cols], AF.Relu,
                                 bias=B2[:], scale=A2[:])
            ov = _view2d(out_sb[:, cols], 44)
            nc.vector.tensor_add(
                ov, ov,
                _view2d(xp2[:, :PG2], 46)[:, 1 + 22 * hh: 23 + 22 * hh, 1:45])
            nc.sync.dma_start(d["d_out"][:, cols], out_sb[:, cols].bitcast(f32))


# ---------------- host-side input prep ----------------

def _prep_shared(inputs):
    import ml_dtypes
    g = lambda k: np.ascontiguousarray(np.asarray(inputs[k], dtype=np.float32))
    w4qk = np.empty((4, 128, 9, 512), np.float32)
    wq, wk = g("w_r4q"), g("w_r4k")       # [256, 512, 3, 3]
    for t in range(4):
        ci = slice(t * 128, (t + 1) * 128)
        # [128ci, 3,3, 256co] per source
        w4qk[t, :, :, :256] = wq[:, ci].transpose(1, 2, 3, 0).reshape(128, 9, 256)
        w4qk[t, :, :, 256:] = wk[:, ci].transpose(1, 2, 3, 0).reshape(128, 9, 256)
    w4qk = w4qk.reshape(4, 128, 9 * 512).astype(ml_dtypes.bfloat16)

    wn3s = g("w_n3")                      # [256, 256, 3, 3]
    wn3 = np.empty((2, 128, 2, 9, 128), np.float32)
    for kt in range(2):
        for ct in range(2):
            blk = wn3s[ct * 128:(ct + 1) * 128, kt * 128:(kt + 1) * 128]
            wn3[kt, :, ct] = blk.transpose(1, 2, 3, 0).reshape(128, 9, 128)
    wn3 = wn3.reshape(2, 128, 2304)
    wn3 = np.concatenate([wn3[0], wn3[1]], axis=1)      # [128, 4608]

    wn2 = g("w_n2").transpose(1, 2, 3, 0).reshape(128, 1152)
    w2r = g("w_2r").transpose(1, 2, 3, 0).reshape(128, 1152)

    w13 = np.empty((2, 128, 2, 3, 128), np.float32)
    for kt in range(2):
        for qk, key in enumerate(("w1_3q", "w1_3k")):
            blk = g(key)[:, kt * 128:(kt + 1) * 128]    # [128co, 128ci, 3]
            w13[kt, :, qk] = blk.transpose(1, 2, 0)
    w13 = np.concatenate([w13[0].reshape(128, 768),
                          w13[1].reshape(128, 768)], axis=1)

    w12 = np.empty((128, 2, 5, 128), np.float32)
    for qk, key in enumerate(("w1_2q", "w1_2k")):
        w12[:, qk] = g(key).transpose(1, 2, 0)
    w12 = w12.reshape(128, 1280)

    bn4 = np.stack([np.concatenate([g("g_r4q"), g("g_r4k")]),
                    np.concatenate([g("be_r4q"), g("be_r4k")])])
    vecs = np.zeros((128, 10), np.float32)
    vecs[:, 0] = g("b_n3")[:128]
    vecs[:, 1] = g("b_n3")[128:]
    vecs[:, 2] = g("b_n2")
    vecs[:, 3] = g("b1_3q")
    vecs[:, 4] = g("b1_3k")
    vecs[:, 5] = g("b1_2q")
    vecs[:, 6] = g("b1_2k")
    vecs[:, 7] = g("g_2r")
    vecs[:, 8] = g("be_2r")
    ones = np.ones((128, 256), np.float32)
    bf = ml_dtypes.bfloat16
    return dict(w4qk=w4qk, wn3=wn3.astype(bf), wn2=wn2.astype(bf),
                w2r=w2r.astype(bf), w13=w13.astype(bf), w12=w12.astype(bf),
                bn4=bn4, vecs=vecs, onesd=ones)


def _prep_sample(inputs, i):
    import ml_dtypes
    bf = ml_dtypes.bfloat16
    x4 = np.asarray(inputs["x4"][i], dtype=np.float32)   # [512, 11, 11]
    x4p = np.zeros((512, 13, 13), np.float32)
    x4p[:, 1:12, 1:12] = x4
    xp4s = np.empty((4, 128, 9, 121), np.float32)
    for t in range(4):
        ci = slice(t * 128, (t + 1) * 128)
        for tap in range(9):
            dy, dx = tap // 3, tap % 3
            xp4s[t, :, tap] = x4p[ci, dy:dy + 11, dx:dx + 11].reshape(128, 121)
    xp4s = xp4s.reshape(4, 128, 9 * 121).astype(bf)

    x3 = np.asarray(inputs["x3"][i], dtype=np.float32)   # [256, 22, 22]
    x3p = np.zeros((256, 24, 24), np.float32)
    x3p[:, 1:23, 1:23] = x3
    xp3 = np.zeros((2, 128, XP3_W), np.float32)
    xp3[:, :, :PG3] = x3p.reshape(2, 128, PG3)
    xp3 = np.concatenate([xp3[0], xp3[1]], axis=1).astype(bf)  # [128, 2*626]

    x2 = np.asarray(inputs["x2"][i], dtype=np.float32)   # [128, 44, 44]
    x2p = np.zeros((128, 46, 46), np.float32)
    x2p[:, 1:45, 1:45] = x2
    xp2 = np.zeros((128, XP2_W), np.float32)
    xp2[:, :PG2] = x2p.reshape(128, PG2)
    return dict(xp4s=xp4s, xp3=xp3, xp2=xp2, xp2b=xp2.astype(bf))


_NC = None
_NC_KEY = None


def _get_nc(shared, debug=False):
    global _NC, _NC_KEY
    key = sum(int(np.abs(np.asarray(v, dtype=np.float32)).sum() * 997)
              for v in shared.values())
    if _NC is None or _NC_KEY != key:
        _NC = build(shared, DEBUG=debug)
        _NC_KEY = key
    return _NC


def run(inputs, trace=False, debug=False):
    shared = _prep_shared(inputs)
    nc = _get_nc(shared, debug)
    in_maps = [_prep_sample(inputs, i) for i in range(N_CORES)]
    last_err = None
    for attempt in range(3):
        try:
            res = run_bass_kernel_spmd(nc, in_maps, list(range(N_CORES)),
                                       trace=trace)
            break
        except Exception as e:  # transient device errors (NRT_EXEC_UNIT etc.)
            last_err = e
            if attempt == 2:
                raise
    out = np.stack([res.results[i]["out"].reshape(128, 44, 44)
                    for i in range(N_CORES)]).astype(np.float32)
    return out, res


def kernel(**inputs):
    out, _ = run(inputs, trace=False)
    return out
